# revision 25
# baseline (speedup 1.0000x reference)
"""Trainium2 Bass kernel for a single-head linear-projection attention block.

Reference computation (B=4, CH=256, N=4096):
    theta = Wt @ x        [B, 32, N]
    phi   = Wp @ x        [B, 32, N]
    g     = Wg @ x        [B, 128, N]
    scores = theta^T phi  [B, N, N]
    beta = softmax(scores, axis=-1)
    attn = g @ beta^T     [B, 128, N]
    out = gamma * (Wo @ attn) + x

Sharding: 8 cores = 4 batches x 2 query-halves. Each core owns one batch's
full sequence (for keys/values) and half the queries. The per-core x is
rotated so its query half is always columns 0:2048, keeping the SPMD program
identical across cores. No collectives are needed.

V3 structure (per core; all matmuls bf16 with fp32 PSUM accumulation):
  - The main loop is ACT(scalar)-bound: every score element must drain
    PSUM->SBUF through one exp ACTIVATE, at (N+352)/clk ns per instruction.
    V3 drains in [128,1536] super-tiles (3 m-tiles x 512 queries) instead of
    [128,1024], amortizing the 352-cycle instruction overhead (~5.5us).
  - Each pass (512 queries) = 10 supers + one [128,1024] pair; passes 1-3
    drain the pair FIRST so the last pass's tail is short; pass 0 uses
    cb-aligned chunks (3+3+2 per 8-m-tile column block) so the drip-fed
    prologue pieces meet their deadlines against x's DMA arrival.
  - softmax denominator: running bf16 chain over drained tiles on DVE
    (validated: S errors wash out through the softmax ratio), folded 1536->512
    at pass end, partition-reduced/broadcast with one ones-matmul.
  - only cb0's projections precede the loop (pipelined around the ~3us
    staggered x kb0/kb1 arrival); cb1-3 projections and all gT halves drip
    into pass 0 just-in-time. theta/phi share one PSUM cast per cb into thph
    (+ a swapped-row-group rep tile via cheap bf16 copies) for 2-way score
    row-group packing.
  - residual reads the bf16 x in SBUF (no fp32 xq DMA); gamma folded into Wo
    on the host; last-pass epilogue in 4 column chunks with PE kept warm.
  - PSUM: 2x[128,1536] score slots (6 banks) + 2x[128,512] psA (attn
    accumulator + epilogue) = 8 banks exactly.
"""

import os
import sys

import numpy as np

B, CH, N = 4, 256, 4096
NCORES = 8
NH = N // 2  # queries per core
P = 128

_REPO_CANDIDATES = ["/opt/trn_rl_repo", "/root/.axon_site/_ro/trn_rl_repo"]


def _ensure_import_path():
    try:
        import concourse.bass  # noqa: F401
        return
    except ImportError:
        pass
    for cand in _REPO_CANDIDATES:
        if os.path.isdir(cand):
            sys.path.insert(0, cand)
            try:
                import concourse.bass  # noqa: F401
                return
            except ImportError:
                sys.path.pop(0)
    raise ImportError("could not locate concourse (bass) repo")


_CACHE = {}


def build_bass():
    """Build + compile the per-core Tile program (identical on all 8 cores)."""
    _ensure_import_path()
    import concourse.bacc as bacc
    import concourse.tile as tile
    from concourse import mybir

    dt = mybir.dt
    f32 = dt.float32
    bf16 = dt.bfloat16
    Exp = mybir.ActivationFunctionType.Exp

    nc = bacc.Bacc(
        "TRN2",
        target_bir_lowering=False,
        debug=False,
        num_devices=NCORES,
    )

    # Per-core DRAM I/O.
    x_d = nc.dram_tensor("x", [CH, N], bf16, kind="ExternalInput")
    wt_d = nc.dram_tensor("wt", [CH, 32], bf16, kind="ExternalInput")   # Wt^T
    wp_d = nc.dram_tensor("wp", [CH, 32], bf16, kind="ExternalInput")   # Wp^T
    wg_d = nc.dram_tensor("wg", [CH, 128], bf16, kind="ExternalInput")  # Wg^T
    wo_d = nc.dram_tensor("wo", [128, CH], bf16, kind="ExternalInput")  # (gamma*Wo)^T
    out_d = nc.dram_tensor("out", [CH, NH], f32, kind="ExternalOutput")

    MT = N // P  # 32 m-tiles
    NQ = 512     # query chunk per pass (4 passes over n)

    # chunk layouts (lists of m-tile lists): pass 0 is cb-aligned 3/3/2 so
    # drips meet x arrival; passes 1-3 put the pair first (short tail)
    chunks0 = []
    for cb in range(4):
        b0 = cb * 8
        chunks0 += [[b0, b0 + 1, b0 + 2], [b0 + 3, b0 + 4, b0 + 5], [b0 + 6, b0 + 7]]
    chunksN = [[30, 31]] + [[3 * s, 3 * s + 1, 3 * s + 2] for s in range(10)]

    with tile.TileContext(nc) as tc:
        with (
            tc.tile_pool(name="const", bufs=1) as const,
            tc.tile_pool(name="xp", bufs=1) as xp,
            tc.tile_pool(name="proj", bufs=1) as proj,
            tc.tile_pool(name="expp", bufs=8) as expp,
            tc.tile_pool(name="acc", bufs=2) as acc,
            tc.tile_pool(name="outp", bufs=1) as outp,
            tc.tile_pool(name="tree", bufs=3) as tree,
            tc.tile_pool(name="ps2", bufs=2, space="PSUM") as ps2,
            tc.tile_pool(name="psA", bufs=2, space="PSUM") as psA,
        ):
            # ---- weights (tiny, gpsimd HWDGE queue) ----
            wt_sb = const.tile([P, 2, 32], bf16)
            wp_sb = const.tile([P, 2, 32], bf16)
            wg_sb = const.tile([P, 2, 128], bf16)
            wo_sb = const.tile([P, CH], bf16)
            ones_sb = const.tile([P, P], bf16)
            nc.gpsimd.dma_start(
                out=wt_sb, in_=wt_d.ap().rearrange("(kb p) m -> p kb m", p=P)
            )
            nc.gpsimd.dma_start(
                out=wp_sb, in_=wp_d.ap().rearrange("(kb p) m -> p kb m", p=P)
            )
            nc.gpsimd.dma_start(
                out=wg_sb, in_=wg_d.ap().rearrange("(kb p) m -> p kb m", p=P)
            )
            nc.gpsimd.dma_start(out=wo_sb, in_=wo_d.ap())
            nc.vector.memset(ones_sb, 1.0)

            warm_sb = const.tile([P, 512], bf16)
            nc.vector.memset(warm_sb, 0.0)

            def emit_warms(k, pool=None, tag="ps"):
                for _ in range(k):
                    ps_w = (pool or ps2).tile([P, 512], f32, tag=tag)
                    nc.tensor.matmul(
                        ps_w, lhsT=ones_sb, rhs=warm_sb, start=True, stop=True
                    )

            # dense dummy matmul burst: trips the PE clock monitor (HAM) to
            # full rate and bridges until x cb0 lands (~14us)
            emit_warms(12)

            # ---- x: kb0 on sync, kb1 on scalar, cb-ordered ----
            x_sb = xp.tile([P, 2, N], bf16)
            for cb in range(4):
                nc.sync.dma_start(
                    out=x_sb[:, 0, cb * 1024:(cb + 1) * 1024],
                    in_=x_d[0:P, cb * 1024:(cb + 1) * 1024],
                )
                nc.scalar.dma_start(
                    out=x_sb[:, 1, cb * 1024:(cb + 1) * 1024],
                    in_=x_d[P:2 * P, cb * 1024:(cb + 1) * 1024],
                )
            # dummy exps: latch the ACT engine's activity-based fast clock
            act_warm = const.tile([P, 512], bf16)
            for _ in range(11):
                nc.scalar.activation(out=act_warm, in_=warm_sb, func=Exp)

            # ---- projections: thph = theta (rows 0:32, cols 0:NH) + phi
            # (rows 32:64); rep = swapped row groups ----
            thph_sb = proj.tile([64, N], bf16)
            rep_sb = proj.tile([64, N], bf16)
            gT_sb = proj.tile([P, MT, P], bf16)

            def emit_proj_mm(cb, kb, ps_p):
                first, last = kb == 0, kb == 1
                for c in range(2):
                    cs = slice(c * 512, (c + 1) * 512)
                    xs = x_sb[:, kb, cb * 1024 + c * 512:cb * 1024 + (c + 1) * 512]
                    if cb < 2:
                        nc.tensor.matmul(
                            ps_p[0:32, cs], lhsT=wt_sb[:, kb, :], rhs=xs,
                            start=first, stop=last, skip_group_check=True,
                        )
                    nc.tensor.matmul(
                        ps_p[32:64, cs], lhsT=wp_sb[:, kb, :], rhs=xs,
                        start=first, stop=last, skip_group_check=True,
                    )

            def emit_proj_cast(cb, ps_p):
                cbs = slice(cb * 1024, (cb + 1) * 1024)
                if cb < 2:
                    nc.vector.tensor_copy(out=thph_sb[0:64, cbs], in_=ps_p)
                    nc.vector.tensor_copy(
                        out=rep_sb[32:64, cbs], in_=thph_sb[0:32, cbs]
                    )
                else:
                    nc.vector.tensor_copy(
                        out=thph_sb[32:64, cbs], in_=ps_p[32:64, :]
                    )
                nc.vector.tensor_copy(out=rep_sb[0:32, cbs], in_=thph_sb[32:64, cbs])

            _proj_ps = {}

            def emit_proj_part(cb, part):
                if part == 0:
                    _proj_ps[cb] = ps2.tile(
                        [64, 1024], f32, tag="ps", name=f"ps_proj{cb}"
                    )
                    emit_proj_mm(cb, 0, _proj_ps[cb])
                else:
                    ps_p = _proj_ps.pop(cb)
                    emit_proj_mm(cb, 1, ps_p)
                    emit_proj_cast(cb, ps_p)

            def emit_gt_half(grp, half):
                ps_g = ps2.tile([P, 4, P], f32, tag="ps")
                for j in range(4):
                    mt = grp * 8 + half * 4 + j
                    for kb in range(2):
                        nc.tensor.matmul(
                            ps_g[:, j, :],
                            lhsT=x_sb[:, kb, mt * P:(mt + 1) * P],
                            rhs=wg_sb[:, kb, :],
                            start=(kb == 0),
                            stop=(kb == 1),
                        )
                base = grp * 8 + half * 4
                nc.vector.tensor_copy(out=gT_sb[:, base:base + 4, :], in_=ps_g)

            # cb0's projection, pipelined around the staggered x arrival
            # (kb0 ~3us before kb1): kb0 matmuls first, warms bridge the PE
            # gap, then per-512-chunk cast+replicas follow the kb1 matmuls
            ps_p0 = ps2.tile([64, 1024], f32, tag="ps")
            emit_proj_mm(0, 0, ps_p0)
            emit_warms(5, pool=psA, tag="attn")
            for c in range(2):
                cs = slice(c * 512, (c + 1) * 512)
                xs = x_sb[:, 1, c * 512:(c + 1) * 512]
                nc.tensor.matmul(
                    ps_p0[0:32, cs], lhsT=wt_sb[:, 1, :], rhs=xs,
                    start=False, stop=True, skip_group_check=True,
                )
                nc.tensor.matmul(
                    ps_p0[32:64, cs], lhsT=wp_sb[:, 1, :], rhs=xs,
                    start=False, stop=True, skip_group_check=True,
                )
                nc.vector.tensor_copy(out=thph_sb[0:64, cs], in_=ps_p0[:, cs])
                nc.vector.tensor_copy(out=rep_sb[0:32, cs], in_=thph_sb[32:64, cs])
                nc.vector.tensor_copy(out=rep_sb[32:64, cs], in_=thph_sb[0:32, cs])
            # first gT half (m0-3) precedes the loop: attn chunk0 reads it
            emit_gt_half(0, 0)

            out_sb = outp.tile([P, 2, NH], f32)

            def epilogue_pieces(nh, attn_ps, S_bf, nchunk, warm_drip=False):
                """Dripped into the next pass (or the program tail)."""
                A_bf = acc.tile([P, NQ], bf16, tag="abf")
                nc.vector.tensor_copy(out=A_bf, in_=attn_ps)
                yield
                ps_S = psA.tile([P, NQ], f32, tag="attn")
                nc.tensor.matmul(ps_S, lhsT=ones_sb, rhs=S_bf, start=True, stop=True)
                yield
                recip = acc.tile([P, NQ], f32, tag="recip")
                nc.vector.reciprocal_approx_fast(out=recip, in_=ps_S)
                yield
                CW = NQ // nchunk
                for ck in range(nchunk):
                    cks = slice(ck * CW, (ck + 1) * CW)
                    gks = slice(nh * NQ + ck * CW, nh * NQ + (ck + 1) * CW)
                    for oc in range(2):
                        ps_o = psA.tile([P, CW], f32, tag="attn")
                        nc.tensor.matmul(
                            ps_o,
                            lhsT=wo_sb[:, oc * P:(oc + 1) * P],
                            rhs=A_bf[:, cks],
                            start=True,
                            stop=True,
                        )
                        tmp = acc.tile([P, CW], f32, tag="tmp")
                        nc.vector.tensor_mul(tmp, ps_o, recip[:, cks])
                        nc.vector.tensor_add(
                            out_sb[:, oc, gks], tmp, x_sb[:, oc, gks]
                        )
                        nc.sync.dma_start(
                            out=out_d[oc * P:(oc + 1) * P, gks],
                            in_=out_sb[:, oc, gks],
                        )
                        if warm_drip:
                            emit_warms(1)
                        yield

            # pass-0 drip schedule, keyed by chunk index (emitted after that
            # chunk's score matmuls). Deadlines: proj cb_k before the chunk
            # whose scores need it; gT half before the chunk whose ATTN reads
            # it (attn trails scores by one chunk).
            drip0 = {
                0: [lambda: emit_gt_half(0, 1), lambda: emit_proj_part(1, 0)],
                1: [lambda: emit_proj_part(1, 1), lambda: emit_gt_half(1, 0)],
                2: [lambda: emit_gt_half(1, 1)],
                4: [lambda: emit_proj_part(2, 0), lambda: emit_gt_half(2, 0)],
                5: [lambda: emit_proj_part(2, 1), lambda: emit_gt_half(2, 1)],
                7: [lambda: emit_proj_part(3, 0), lambda: emit_gt_half(3, 0)],
                8: [lambda: emit_proj_part(3, 1), lambda: emit_gt_half(3, 1)],
            }

            pending = None
            for nh in range(4):
                chunks = chunks0 if nh == 0 else chunksN
                ns = slice(nh * NQ, (nh + 1) * NQ)
                attn_ps = psA.tile([P, NQ], f32, tag="attn")
                chain = None          # running bf16 sum of 1536-wide supers
                pair_fold = None      # folded [P,512] of the pair chunk
                n_mm = 0
                for ci, mts in enumerate(chunks):
                    W = 512 * len(mts)
                    ps_s = ps2.tile([P, 1536], f32, tag="ps")
                    for k, mt in enumerate(mts):
                        j = mt % 2
                        lhsT = (
                            rep_sb[0:32, mt * P:(mt + 1) * P]
                            if j == 0
                            else thph_sb[32:64, mt * P:(mt + 1) * P]
                        )
                        rhs = thph_sb[0:32, ns] if j == 0 else rep_sb[32:64, ns]
                        nc.tensor.matmul(
                            ps_s[:, k * 512:(k + 1) * 512],
                            lhsT=lhsT,
                            rhs=rhs,
                            start=True,
                            stop=True,
                            skip_group_check=True,
                        )
                    if nh == 0 and ci in drip0:
                        for fn in drip0[ci]:
                            fn()
                    expt = expp.tile([P, 1536], bf16, tag="expt")
                    nc.scalar.activation(
                        out=expt[:, 0:W], in_=ps_s[:, 0:W], func=Exp
                    )
                    for k, mt in enumerate(mts):
                        nc.tensor.matmul(
                            attn_ps,
                            lhsT=gT_sb[:, mt, :],
                            rhs=expt[:, k * 512:(k + 1) * 512],
                            start=(n_mm == 0 and k == 0),
                            stop=(n_mm + k == MT - 1),
                            skip_group_check=True,
                        )
                    n_mm += len(mts)
                    # S accumulation on DVE (bf16, validated)
                    if len(mts) == 3:
                        if chain is None:
                            chain = expt
                        else:
                            nt = tree.tile([P, 1536], bf16, tag="chain")
                            nc.vector.tensor_add(nt, chain, expt[:, 0:1536])
                            chain = nt
                    else:  # pair chunk: fold to [P,512] immediately
                        pf = tree.tile([P, NQ], bf16, tag="pfold")
                        nc.vector.tensor_add(pf, expt[:, 0:512], expt[:, 512:1024])
                        if pair_fold is None:
                            pair_fold = pf
                        else:
                            pf2 = tree.tile([P, NQ], bf16, tag="pfold2")
                            nc.vector.tensor_add(pf2, pair_fold, pf)
                            pair_fold = pf2
                    if pending is not None and ci % 2 == 1:
                        next(pending, None)
                # fold chain 1536 -> 512 and merge the pair fold
                f1 = tree.tile([P, NQ], bf16, tag="f1")
                nc.vector.tensor_add(f1, chain[:, 0:512], chain[:, 512:1024])
                f2 = tree.tile([P, NQ], bf16, tag="f2")
                nc.vector.tensor_add(f2, f1, chain[:, 1024:1536])
                S_bf = tree.tile([P, NQ], bf16, tag="sfold")
                nc.vector.tensor_add(S_bf, f2, pair_fold)
                if pending is not None:
                    for _ in pending:
                        pass
                pending = epilogue_pieces(
                    nh, attn_ps, S_bf,
                    nchunk=1 if nh < 3 else 4,
                    warm_drip=(nh == 3),
                )
            for _ in pending:
                pass

    nc.compile()
    return nc


def get_nc():
    if "nc" not in _CACHE:
        _CACHE["nc"] = build_bass()
    return _CACHE["nc"]


def make_in_maps(x, Wt, Wp, Wg, Wo, gamma):
    import ml_dtypes

    bf16 = ml_dtypes.bfloat16
    x = np.asarray(x, dtype=np.float32)
    wt = np.ascontiguousarray(np.asarray(Wt, np.float32).T).astype(bf16)
    wp = np.ascontiguousarray(np.asarray(Wp, np.float32).T).astype(bf16)
    wg = np.ascontiguousarray(np.asarray(Wg, np.float32).T).astype(bf16)
    wo = np.ascontiguousarray(
        (float(np.asarray(gamma)) * np.asarray(Wo, np.float32)).T
    ).astype(bf16)
    in_maps = []
    for i in range(NCORES):
        b, h = divmod(i, 2)
        xb = x[b]
        if h:
            xb = np.concatenate([xb[:, NH:], xb[:, :NH]], axis=1)
        in_maps.append(
            {
                "x": np.ascontiguousarray(xb).astype(bf16),
                "wt": wt,
                "wp": wp,
                "wg": wg,
                "wo": wo,
            }
        )
    return in_maps


def gather_out(results):
    out = np.empty((B, CH, N), np.float32)
    for i in range(NCORES):
        b, h = divmod(i, 2)
        out[b][:, h * NH:(h + 1) * NH] = results[i]["out"]
    return out


def kernel(x, Wt, Wp, Wg, Wo, gamma):
    _ensure_import_path()
    from concourse.bass_utils import run_bass_kernel_spmd

    nc = get_nc()
    in_maps = make_in_maps(x, Wt, Wp, Wg, Wo, gamma)
    res = run_bass_kernel_spmd(nc, in_maps, core_ids=list(range(NCORES)))
    return gather_out(res.results)
